# revision 20
# baseline (speedup 1.0000x reference)
"""ChebyKAN layer on 8 TRN2 NeuronCores (data-parallel over batch).

y[b,o] = sum_{i,d} T_d(tanh(x[b,i])) * C[i,o,d],  d = 0..8

Device algorithm (per core, batch shard of 2048 rows):
  - T_0 = 1 is folded into a host-computed bias: bias[o] = sum_i C[i,o,0].
  - t = tanh(x) on ACT; T_d via Chebyshev recurrence T_{n+1} = 2 t T_n - T_{n-1}
    on DVE in fp32 (basis laid out transposed: [i_chunk=128 part, batch free]).
  - Basis tiles rounded to float32r (tf32-like: 1 cycle/row on the PE at
    free dim >= 256, ~11 mantissa bits) and used as matmul stationary;
    coefficient chunks [i=128, o=512] are the moving operand, accumulated
    over (d, i_chunk) into PSUM [b=128, o=512].
  - PSUM evacuated with a fused bias add on DVE, stored to DRAM.

Inputs arrive FULL; sharding/transpose/reorder happen on the host here.
"""

import numpy as np

import concourse.bacc as bacc
import concourse.tile as tile
from concourse import mybir
from concourse.bass_utils import run_bass_kernel_spmd

dt = mybir.dt

BATCH = 16384
I_DIM = 512
O_DIM = 512
DEG = 8            # d = 1..8 on device; d=0 via bias
N_CORES = 8
B_CORE = BATCH // N_CORES      # 2048
B_BLK = 512                    # batch rows per block
N_BLK = B_CORE // B_BLK        # 4
N_IC = I_DIM // 128            # 4 input chunks
N_BS = B_BLK // 128            # 4 psum row-tiles per block

_CACHE = {}


def _build_program():
    from contextlib import ExitStack

    AF = mybir.ActivationFunctionType
    OP = mybir.AluOpType

    nc = bacc.Bacc(num_swdge_queues=4)
    xt_in = nc.declare_dram_parameter("xt", [I_DIM, B_CORE], dt.float32, isOutput=False)
    cd_in = nc.declare_dram_parameter("cd", [DEG, I_DIM, O_DIM], dt.float32, isOutput=False)
    bias_in = nc.declare_dram_parameter("bias", [1, O_DIM], dt.float32, isOutput=False)
    ones_in = nc.declare_dram_parameter("ones", [1, 128], dt.float32, isOutput=False)
    y_out = nc.declare_dram_parameter("y", [B_CORE, O_DIM], dt.float32, isOutput=True)

    # Two i-chunks are batched per elementwise op: every chain op works
    # on [128, 2*B_BLK] = [128, 1024].  Degrees live in slots:
    #   stage A slots: 0:t 1:T2 2:T3 3:T4   -> cast A
    #   stage B slots: 0:T5 1:T6 2:T7 3:T8  -> cast B
    PW = 2 * B_BLK            # 1024, pair width
    DEG_A = {1: 0, 2: 1, 3: 2, 4: 3}
    DEG_B = {5: 0, 6: 1, 7: 2, 8: 3}

    with tile.TileContext(nc) as tc, ExitStack() as ctx:
        cpool = ctx.enter_context(tc.tile_pool(name="cpool", bufs=1))
        xpool = ctx.enter_context(tc.tile_pool(name="xpool", bufs=2))
        fpool = ctx.enter_context(tc.tile_pool(name="fpool", bufs=2))
        rpool = ctx.enter_context(tc.tile_pool(name="rpool", bufs=3))
        mvpool = ctx.enter_context(tc.tile_pool(name="mvpool", bufs=2))
        s2pool = ctx.enter_context(tc.tile_pool(name="s2pool", bufs=1))
        mgpool = ctx.enter_context(tc.tile_pool(name="mgpool", bufs=2))
        opool = ctx.enter_context(tc.tile_pool(name="opool", bufs=2))
        pspool = ctx.enter_context(tc.tile_pool(name="pspool", bufs=8, space="PSUM"))

        # Bias (T_0 term) and a ones row: K=1 matmul seeds PSUM with the bias.
        bias_t = cpool.tile([1, O_DIM], dt.float16, tag="bias")
        nc.gpsimd.dma_start(out=bias_t[:], in_=bias_in[:])
        ones_t = cpool.tile([1, 128], dt.float16, tag="ones")
        nc.gpsimd.dma_start(out=ones_t[:], in_=ones_in[:])

        # Coefficients: one wide cast-DMA (fp32 -> f32r) per degree, resident.
        c_tiles = {}
        for d in range(DEG):
            c = cpool.tile([128, N_IC, O_DIM], dt.float16, tag=f"c{d}", name=f"c{d}")
            nc.gpsimd.dma_start(
                out=c[:],
                in_=cd_in[d].rearrange("(ic p) o -> p ic o", p=128),
            )
            c_tiles[d] = c

        # PE warm-up: dummy matmuls keep the HAM clock-gate open while the
        # first basis tiles are being computed.
        warm = pspool.tile([128, 128], dt.float32, tag="ps", name="warm")
        for _ in range(64):
            nc.tensor.matmul(warm[:], lhsT=ones_t[:], rhs=bias_t[:, 0:128],
                             start=True, stop=True)

        for blk in range(N_BLK):
            b0 = blk * B_BLK
            ps = []
            for bs in range(N_BS):
                p = pspool.tile([128, O_DIM], dt.float32, tag="ps", name="ps")
                nc.tensor.matmul(
                    p[:], lhsT=ones_t[:], rhs=bias_t[:], start=True, stop=False
                )
                ps.append(p)
            for pair in range(N_IC // 2):
                ic0 = pair * 2
                xt = xpool.tile([128, PW], dt.float32, tag="xt")
                for h in range(2):
                    ic = ic0 + h
                    nc.sync.dma_start(
                        out=xt[:, h * B_BLK:(h + 1) * B_BLK],
                        in_=xt_in[ic * 128:(ic + 1) * 128, b0:b0 + B_BLK],
                    )
                FA = fpool.tile([128, 4 * PW], dt.float32, tag="FA", name="FA")
                FB = fpool.tile([128, 4 * PW], dt.float32, tag="FB", name="FB")

                def sa(i):
                    return FA[:, i * PW:(i + 1) * PW]

                def sb(i):
                    return FB[:, i * PW:(i + 1) * PW]

                t, s, T3, T4 = sa(0), sa(1), sa(2), sa(3)
                T5, T6, T7, T8 = sb(0), sb(1), sb(2), sb(3)

                nc.scalar.activation(t, xt[:], AF.Tanh)

                # DVE: T2, preps, even chain (tensor_scalar runs in 2x mode)
                m2 = mvpool.tile([128, PW], dt.float32, tag="mv", name="m2")
                nc.vector.scalar_tensor_tensor(m2[:], t, 2.0, t, OP.mult, OP.mult)
                nc.vector.tensor_scalar_sub(s, m2[:], 1.0)
                s2 = s2pool.tile([128, PW], dt.float32, tag="s2", name="s2")
                nc.vector.tensor_scalar_mul(s2[:], s, 2.0)
                w = s2pool.tile([128, PW], dt.float32, tag="w", name="w")
                nc.vector.tensor_scalar(w[:], s, 2.0, 1.0, OP.mult, OP.subtract)
                m4 = mvpool.tile([128, PW], dt.float32, tag="mv", name="m4")
                nc.vector.scalar_tensor_tensor(m4[:], s, 2.0, s, OP.mult, OP.mult)
                nc.vector.tensor_scalar_sub(T4, m4[:], 1.0)
                m6 = mvpool.tile([128, PW], dt.float32, tag="mv", name="m6")
                nc.vector.scalar_tensor_tensor(m6[:], T4, 2.0, s, OP.mult, OP.mult)
                nc.vector.tensor_sub(T6, m6[:], s)
                m8 = mvpool.tile([128, PW], dt.float32, tag="mv", name="m8")
                nc.vector.scalar_tensor_tensor(m8[:], T6, 2.0, s, OP.mult, OP.mult)
                nc.vector.tensor_sub(T8, m8[:], T4)

                # GpSimd: odd chain muls; final T7 subtract on DVE
                nc.gpsimd.tensor_mul(T3, t, w[:])
                m5 = mgpool.tile([128, PW], dt.float32, tag="mg", name="m5")
                nc.gpsimd.tensor_mul(m5[:], s2[:], T3)
                nc.gpsimd.tensor_sub(T5, m5[:], t)
                m7 = mgpool.tile([128, PW], dt.float32, tag="mg", name="m7")
                nc.gpsimd.tensor_mul(m7[:], s2[:], T5)
                nc.gpsimd.tensor_sub(T7, m7[:], T3)

                # Two-stage rounding casts fp32 -> f32r on ACT.
                RA = rpool.tile([128, 4 * PW], dt.float16, tag="RA", name="RA")
                nc.scalar.activation(RA[:], FA[:], AF.Copy)
                RB = rpool.tile([128, 4 * PW], dt.float16, tag="RB", name="RB")
                nc.scalar.activation(RB[:], FB[:], AF.Copy)

                # Matmuls: stage-A degrees first (overlap with cast B).
                for stage, R, degs in (("A", RA, DEG_A), ("B", RB, DEG_B)):
                    for h in range(2):
                        ic = ic0 + h
                        for bs in range(N_BS):
                            for d, slot in degs.items():
                                nc.tensor.matmul(
                                    ps[bs][:],
                                    lhsT=R[:, slot * PW + h * B_BLK + bs * 128:
                                           slot * PW + h * B_BLK + (bs + 1) * 128],
                                    rhs=c_tiles[d - 1][:, ic, :],
                                    start=False,
                                    stop=(pair == 1 and stage == "B"
                                          and h == 1 and d == DEG),
                                )

            for bs in range(N_BS):
                o = opool.tile([128, O_DIM], dt.float32, tag="o")
                nc.scalar.activation(o[:], ps[bs][:], AF.Copy)
                nc.sync.dma_start(
                    out=y_out[b0 + bs * 128: b0 + (bs + 1) * 128, :], in_=o[:]
                )

    nc.compile()
    return nc


def _get_program():
    if "nc" not in _CACHE:
        _CACHE["nc"] = _build_program()
    return _CACHE["nc"]


def _prep_inputs(x, cheby_coeffs):
    x = np.ascontiguousarray(x, dtype=np.float32)
    c = np.ascontiguousarray(cheby_coeffs, dtype=np.float32)
    cd = np.ascontiguousarray(np.transpose(c, (2, 0, 1))[1:DEG + 1])  # [8, I, O]
    bias = c[:, :, 0].sum(axis=0, dtype=np.float64).astype(np.float32)[None, :]
    ones = np.ones((1, 128), dtype=np.float32)
    in_maps = []
    for core in range(N_CORES):
        xs = x[core * B_CORE:(core + 1) * B_CORE]          # [2048, I]
        xt = np.ascontiguousarray(xs.T)                     # [I, 2048]
        in_maps.append({"xt": xt, "cd": cd, "bias": bias, "ones": ones})
    return in_maps


def run(x, cheby_coeffs, trace=False, **trace_kwargs):
    nc = _get_program()
    in_maps = _prep_inputs(x, cheby_coeffs)
    res = run_bass_kernel_spmd(
        nc, in_maps, list(range(N_CORES)), trace=trace, **trace_kwargs
    )
    y = np.concatenate([res.results[i]["y"] for i in range(N_CORES)], axis=0)
    return y, res


def kernel(x, cheby_coeffs):
    y, _ = run(x, cheby_coeffs)
    return y


# revision 21
# speedup vs baseline: 1.0063x; 1.0063x over previous
"""ChebyKAN layer on 8 TRN2 NeuronCores (data-parallel over batch).

y[b,o] = sum_{i,d} T_d(tanh(x[b,i])) * C[i,o,d],  d = 0..8

Device algorithm (per core, batch shard of 2048 rows):
  - T_0 = 1 is folded into a host-computed bias: bias[o] = sum_i C[i,o,0].
  - t = tanh(x) on ACT; T_d via Chebyshev recurrence T_{n+1} = 2 t T_n - T_{n-1}
    on DVE in fp32 (basis laid out transposed: [i_chunk=128 part, batch free]).
  - Basis tiles rounded to float32r (tf32-like: 1 cycle/row on the PE at
    free dim >= 256, ~11 mantissa bits) and used as matmul stationary;
    coefficient chunks [i=128, o=512] are the moving operand, accumulated
    over (d, i_chunk) into PSUM [b=128, o=512].
  - PSUM evacuated with a fused bias add on DVE, stored to DRAM.

Inputs arrive FULL; sharding/transpose/reorder happen on the host here.
"""

import numpy as np

import concourse.bacc as bacc
import concourse.tile as tile
from concourse import mybir
from concourse.bass_utils import run_bass_kernel_spmd

dt = mybir.dt

BATCH = 16384
I_DIM = 512
O_DIM = 512
DEG = 8            # d = 1..8 on device; d=0 via bias
N_CORES = 8
B_CORE = BATCH // N_CORES      # 2048
B_BLK = 512                    # batch rows per block
N_BLK = B_CORE // B_BLK        # 4
N_IC = I_DIM // 128            # 4 input chunks
N_BS = B_BLK // 128            # 4 psum row-tiles per block

_CACHE = {}


def _build_program():
    from contextlib import ExitStack

    AF = mybir.ActivationFunctionType
    OP = mybir.AluOpType

    nc = bacc.Bacc(num_swdge_queues=4)
    xt_in = nc.declare_dram_parameter("xt", [I_DIM, B_CORE], dt.float32, isOutput=False)
    cd_in = nc.declare_dram_parameter("cd", [DEG, I_DIM, O_DIM], dt.float32, isOutput=False)
    bias_in = nc.declare_dram_parameter("bias", [1, O_DIM], dt.float32, isOutput=False)
    ones_in = nc.declare_dram_parameter("ones", [1, 128], dt.float32, isOutput=False)
    y_out = nc.declare_dram_parameter("y", [B_CORE, O_DIM], dt.float32, isOutput=True)

    # Two i-chunks are batched per elementwise op: every chain op works
    # on [128, 2*B_BLK] = [128, 1024].  Degrees live in slots:
    #   stage A slots: 0:t 1:T2 2:T3 3:T4   -> cast A
    #   stage B slots: 0:T5 1:T6 2:T7 3:T8  -> cast B
    PW = 2 * B_BLK            # 1024, pair width
    DEG_A = {1: 0, 2: 1, 3: 2, 4: 3}
    DEG_B = {5: 0, 6: 1, 7: 2, 8: 3}

    with tile.TileContext(nc) as tc, ExitStack() as ctx:
        cpool = ctx.enter_context(tc.tile_pool(name="cpool", bufs=1))
        xpool = ctx.enter_context(tc.tile_pool(name="xpool", bufs=2))
        fpool = ctx.enter_context(tc.tile_pool(name="fpool", bufs=2))
        rpool = ctx.enter_context(tc.tile_pool(name="rpool", bufs=3))
        mvpool = ctx.enter_context(tc.tile_pool(name="mvpool", bufs=2))
        s2pool = ctx.enter_context(tc.tile_pool(name="s2pool", bufs=1))
        mgpool = ctx.enter_context(tc.tile_pool(name="mgpool", bufs=2))
        opool = ctx.enter_context(tc.tile_pool(name="opool", bufs=2))
        pspool = ctx.enter_context(tc.tile_pool(name="pspool", bufs=8, space="PSUM"))

        # Bias (T_0 term) and a ones row: K=1 matmul seeds PSUM with the bias.
        bias_t = cpool.tile([1, O_DIM], dt.float16, tag="bias")
        nc.gpsimd.dma_start(out=bias_t[:], in_=bias_in[:])
        ones_t = cpool.tile([1, 128], dt.float16, tag="ones")
        nc.gpsimd.dma_start(out=ones_t[:], in_=ones_in[:])

        # Coefficients: one wide cast-DMA (fp32 -> f32r) per degree, resident.
        c_tiles = {}
        for d in range(DEG):
            c = cpool.tile([128, N_IC, O_DIM], dt.float16, tag=f"c{d}", name=f"c{d}")
            nc.gpsimd.dma_start(
                out=c[:],
                in_=cd_in[d].rearrange("(ic p) o -> p ic o", p=128),
            )
            c_tiles[d] = c

        for blk in range(N_BLK):
            b0 = blk * B_BLK
            ps = []
            for bs in range(N_BS):
                p = pspool.tile([128, O_DIM], dt.float32, tag="ps", name="ps")
                nc.tensor.matmul(
                    p[:], lhsT=ones_t[:], rhs=bias_t[:], start=True, stop=False
                )
                ps.append(p)
            for pair in range(N_IC // 2):
                ic0 = pair * 2
                xt = xpool.tile([128, PW], dt.float32, tag="xt")
                for h in range(2):
                    ic = ic0 + h
                    nc.sync.dma_start(
                        out=xt[:, h * B_BLK:(h + 1) * B_BLK],
                        in_=xt_in[ic * 128:(ic + 1) * 128, b0:b0 + B_BLK],
                    )
                FA = fpool.tile([128, 4 * PW], dt.float32, tag="FA", name="FA")
                FB = fpool.tile([128, 4 * PW], dt.float32, tag="FB", name="FB")

                def sa(i):
                    return FA[:, i * PW:(i + 1) * PW]

                def sb(i):
                    return FB[:, i * PW:(i + 1) * PW]

                t, s, T3, T4 = sa(0), sa(1), sa(2), sa(3)
                T5, T6, T7, T8 = sb(0), sb(1), sb(2), sb(3)

                nc.scalar.activation(t, xt[:], AF.Tanh)

                # DVE: T2, preps, even chain (tensor_scalar runs in 2x mode)
                m2 = mvpool.tile([128, PW], dt.float32, tag="mv", name="m2")
                nc.vector.scalar_tensor_tensor(m2[:], t, 2.0, t, OP.mult, OP.mult)
                nc.vector.tensor_scalar_sub(s, m2[:], 1.0)
                s2 = s2pool.tile([128, PW], dt.float32, tag="s2", name="s2")
                nc.vector.tensor_scalar_mul(s2[:], s, 2.0)
                w = s2pool.tile([128, PW], dt.float32, tag="w", name="w")
                nc.vector.tensor_scalar(w[:], s, 2.0, 1.0, OP.mult, OP.subtract)
                m4 = mvpool.tile([128, PW], dt.float32, tag="mv", name="m4")
                nc.vector.scalar_tensor_tensor(m4[:], s, 2.0, s, OP.mult, OP.mult)
                nc.vector.tensor_scalar_sub(T4, m4[:], 1.0)
                m6 = mvpool.tile([128, PW], dt.float32, tag="mv", name="m6")
                nc.vector.scalar_tensor_tensor(m6[:], T4, 2.0, s, OP.mult, OP.mult)
                nc.vector.tensor_sub(T6, m6[:], s)
                m8 = mvpool.tile([128, PW], dt.float32, tag="mv", name="m8")
                nc.vector.scalar_tensor_tensor(m8[:], T6, 2.0, s, OP.mult, OP.mult)
                nc.vector.tensor_sub(T8, m8[:], T4)

                # GpSimd: odd chain muls; final T7 subtract on DVE
                nc.gpsimd.tensor_mul(T3, t, w[:])
                m5 = mgpool.tile([128, PW], dt.float32, tag="mg", name="m5")
                nc.gpsimd.tensor_mul(m5[:], s2[:], T3)
                nc.gpsimd.tensor_sub(T5, m5[:], t)
                m7 = mgpool.tile([128, PW], dt.float32, tag="mg", name="m7")
                nc.gpsimd.tensor_mul(m7[:], s2[:], T5)
                nc.gpsimd.tensor_sub(T7, m7[:], T3)

                # Two-stage rounding casts fp32 -> fp16 on ACT.
                RA = rpool.tile([128, 4 * PW], dt.float16, tag="RA", name="RA")
                nc.scalar.activation(RA[:], FA[:], AF.Copy)
                RB = rpool.tile([128, 4 * PW], dt.float16, tag="RB", name="RB")
                nc.scalar.activation(RB[:], FB[:], AF.Copy)

                # Matmuls: stage-A degrees first (overlap with cast B).
                for stage, R, degs in (("A", RA, DEG_A), ("B", RB, DEG_B)):
                    for h in range(2):
                        ic = ic0 + h
                        for bs in range(N_BS):
                            for d, slot in degs.items():
                                nc.tensor.matmul(
                                    ps[bs][:],
                                    lhsT=R[:, slot * PW + h * B_BLK + bs * 128:
                                           slot * PW + h * B_BLK + (bs + 1) * 128],
                                    rhs=c_tiles[d - 1][:, ic, :],
                                    start=False,
                                    stop=(pair == 1 and stage == "B"
                                          and h == 1 and d == DEG),
                                )

            for bs in range(N_BS):
                o = opool.tile([128, O_DIM], dt.float32, tag="o")
                nc.scalar.activation(o[:], ps[bs][:], AF.Copy)
                nc.sync.dma_start(
                    out=y_out[b0 + bs * 128: b0 + (bs + 1) * 128, :], in_=o[:]
                )

    nc.compile()
    return nc


def _get_program():
    if "nc" not in _CACHE:
        _CACHE["nc"] = _build_program()
    return _CACHE["nc"]


def _prep_inputs(x, cheby_coeffs):
    x = np.ascontiguousarray(x, dtype=np.float32)
    c = np.ascontiguousarray(cheby_coeffs, dtype=np.float32)
    cd = np.ascontiguousarray(np.transpose(c, (2, 0, 1))[1:DEG + 1])  # [8, I, O]
    bias = c[:, :, 0].sum(axis=0, dtype=np.float64).astype(np.float32)[None, :]
    ones = np.ones((1, 128), dtype=np.float32)
    in_maps = []
    for core in range(N_CORES):
        xs = x[core * B_CORE:(core + 1) * B_CORE]          # [2048, I]
        xt = np.ascontiguousarray(xs.T)                     # [I, 2048]
        in_maps.append({"xt": xt, "cd": cd, "bias": bias, "ones": ones})
    return in_maps


def run(x, cheby_coeffs, trace=False, **trace_kwargs):
    nc = _get_program()
    in_maps = _prep_inputs(x, cheby_coeffs)
    res = run_bass_kernel_spmd(
        nc, in_maps, list(range(N_CORES)), trace=trace, **trace_kwargs
    )
    y = np.concatenate([res.results[i]["y"] for i in range(N_CORES)], axis=0)
    return y, res


def kernel(x, cheby_coeffs):
    y, _ = run(x, cheby_coeffs)
    return y


# revision 22
# speedup vs baseline: 1.0361x; 1.0296x over previous
"""ChebyKAN layer on 8 TRN2 NeuronCores (data-parallel over batch).

y[b,o] = sum_{i,d} T_d(tanh(x[b,i])) * C[i,o,d],  d = 0..8

Device algorithm (per core, batch shard of 2048 rows):
  - T_0 = 1 is folded into a host-computed bias: bias[o] = sum_i C[i,o,0].
  - t = tanh(x) on ACT; T_d via Chebyshev recurrence T_{n+1} = 2 t T_n - T_{n-1}
    on DVE in fp32 (basis laid out transposed: [i_chunk=128 part, batch free]).
  - Basis tiles rounded to float32r (tf32-like: 1 cycle/row on the PE at
    free dim >= 256, ~11 mantissa bits) and used as matmul stationary;
    coefficient chunks [i=128, o=512] are the moving operand, accumulated
    over (d, i_chunk) into PSUM [b=128, o=512].
  - PSUM evacuated with a fused bias add on DVE, stored to DRAM.

Inputs arrive FULL; sharding/transpose/reorder happen on the host here.
"""

import numpy as np

import concourse.bacc as bacc
import concourse.tile as tile
from concourse import mybir
from concourse.bass_utils import run_bass_kernel_spmd

dt = mybir.dt

BATCH = 16384
I_DIM = 512
O_DIM = 512
DEG = 8            # d = 1..8 on device; d=0 via bias
N_CORES = 8
B_CORE = BATCH // N_CORES      # 2048
B_BLK = 512                    # batch rows per block
N_BLK = B_CORE // B_BLK        # 4
N_IC = I_DIM // 128            # 4 input chunks
N_BS = B_BLK // 128            # 4 psum row-tiles per block

_CACHE = {}


def _build_program():
    from contextlib import ExitStack

    AF = mybir.ActivationFunctionType
    OP = mybir.AluOpType

    nc = bacc.Bacc(num_swdge_queues=4)
    xt_in = nc.declare_dram_parameter("xt", [I_DIM, B_CORE], dt.float32, isOutput=False)
    cd_in = nc.declare_dram_parameter("cd", [DEG, I_DIM, O_DIM], dt.float32, isOutput=False)
    bias_in = nc.declare_dram_parameter("bias", [1, O_DIM], dt.float32, isOutput=False)
    ones_in = nc.declare_dram_parameter("ones", [1, 128], dt.float32, isOutput=False)
    y_out = nc.declare_dram_parameter("y", [B_CORE, O_DIM], dt.float32, isOutput=True)

    # Two i-chunks are batched per elementwise op: every chain op works
    # on [128, 2*B_BLK] = [128, 1024].  Degrees live in slots:
    #   stage A slots: 0:t 1:T2 2:T3 3:T4   -> cast A
    #   stage B slots: 0:T5 1:T6 2:T7 3:T8  -> cast B
    PW = 2 * B_BLK            # 1024, pair width
    DEG_A = {1: 0, 2: 1, 3: 2, 4: 3}
    DEG_B = {5: 0, 6: 1, 7: 2, 8: 3}

    with tile.TileContext(nc) as tc, ExitStack() as ctx:
        cpool = ctx.enter_context(tc.tile_pool(name="cpool", bufs=1))
        xpool = ctx.enter_context(tc.tile_pool(name="xpool", bufs=2))
        fpool = ctx.enter_context(tc.tile_pool(name="fpool", bufs=2))
        rpool = ctx.enter_context(tc.tile_pool(name="rpool", bufs=2))
        mvpool = ctx.enter_context(tc.tile_pool(name="mvpool", bufs=2))
        s2pool = ctx.enter_context(tc.tile_pool(name="s2pool", bufs=1))
        mgpool = ctx.enter_context(tc.tile_pool(name="mgpool", bufs=2))
        opool = ctx.enter_context(tc.tile_pool(name="opool", bufs=2))
        pspool = ctx.enter_context(tc.tile_pool(name="pspool", bufs=8, space="PSUM"))

        # Bias (T_0 term) and a ones row: K=1 matmul seeds PSUM with the bias.
        bias_t = cpool.tile([1, O_DIM], dt.float16, tag="bias")
        nc.gpsimd.dma_start(out=bias_t[:], in_=bias_in[:])
        ones_t = cpool.tile([1, 128], dt.float16, tag="ones")
        nc.gpsimd.dma_start(out=ones_t[:], in_=ones_in[:])

        # Coefficients: one wide cast-DMA (fp32 -> f32r) per degree, resident.
        c_tiles = {}
        for d in range(DEG):
            c = cpool.tile([128, N_IC, O_DIM], dt.float16, tag=f"c{d}", name=f"c{d}")
            nc.gpsimd.dma_start(
                out=c[:],
                in_=cd_in[d].rearrange("(ic p) o -> p ic o", p=128),
            )
            c_tiles[d] = c

        for blk in range(N_BLK):
            b0 = blk * B_BLK
            ps = []
            for bs in range(N_BS):
                p = pspool.tile([128, O_DIM], dt.float32, tag="ps", name="ps")
                nc.tensor.matmul(
                    p[:], lhsT=ones_t[:], rhs=bias_t[:], start=True, stop=False
                )
                ps.append(p)
            for pair in range(N_IC // 2):
                ic0 = pair * 2
                xt = xpool.tile([128, PW], dt.float32, tag="xt")
                for h in range(2):
                    ic = ic0 + h
                    nc.sync.dma_start(
                        out=xt[:, h * B_BLK:(h + 1) * B_BLK],
                        in_=xt_in[ic * 128:(ic + 1) * 128, b0:b0 + B_BLK],
                    )
                FA = fpool.tile([128, 4 * PW], dt.float32, tag="FA", name="FA")
                FB = fpool.tile([128, 4 * PW], dt.float32, tag="FB", name="FB")

                def sa(i):
                    return FA[:, i * PW:(i + 1) * PW]

                def sb(i):
                    return FB[:, i * PW:(i + 1) * PW]

                t, s, T3, T4 = sa(0), sa(1), sa(2), sa(3)
                T5, T6, T7, T8 = sb(0), sb(1), sb(2), sb(3)

                nc.scalar.activation(t, xt[:], AF.Tanh)

                # DVE: T2, preps, even chain (tensor_scalar runs in 2x mode)
                m2 = mvpool.tile([128, PW], dt.float32, tag="mv", name="m2")
                nc.vector.scalar_tensor_tensor(m2[:], t, 2.0, t, OP.mult, OP.mult)
                nc.vector.tensor_scalar_sub(s, m2[:], 1.0)
                s2 = s2pool.tile([128, PW], dt.float32, tag="s2", name="s2")
                nc.vector.tensor_scalar_mul(s2[:], s, 2.0)
                w = s2pool.tile([128, PW], dt.float32, tag="w", name="w")
                nc.vector.tensor_scalar(w[:], s, 2.0, 1.0, OP.mult, OP.subtract)
                m4 = mvpool.tile([128, PW], dt.float32, tag="mv", name="m4")
                nc.vector.scalar_tensor_tensor(m4[:], s, 2.0, s, OP.mult, OP.mult)
                nc.vector.tensor_scalar_sub(T4, m4[:], 1.0)
                m6 = mvpool.tile([128, PW], dt.float32, tag="mv", name="m6")
                nc.vector.scalar_tensor_tensor(m6[:], T4, 2.0, s, OP.mult, OP.mult)
                nc.vector.tensor_sub(T6, m6[:], s)
                m8 = mvpool.tile([128, PW], dt.float32, tag="mv", name="m8")
                nc.vector.scalar_tensor_tensor(m8[:], T6, 2.0, s, OP.mult, OP.mult)
                nc.vector.tensor_sub(T8, m8[:], T4)

                # GpSimd: odd chain muls; final T7 subtract on DVE
                nc.gpsimd.tensor_mul(T3, t, w[:])
                m5 = mgpool.tile([128, PW], dt.float32, tag="mg", name="m5")
                nc.gpsimd.tensor_mul(m5[:], s2[:], T3)
                nc.gpsimd.tensor_sub(T5, m5[:], t)
                m7 = mgpool.tile([128, PW], dt.float32, tag="mg", name="m7")
                nc.gpsimd.tensor_mul(m7[:], s2[:], T5)
                nc.gpsimd.tensor_sub(T7, m7[:], T3)

                # Two-stage rounding casts fp32 -> fp16 on ACT.
                RA = rpool.tile([128, 4 * PW], dt.float16, tag="RA", name="RA")
                nc.scalar.activation(RA[:], FA[:], AF.Copy)
                RB = rpool.tile([128, 4 * PW], dt.float16, tag="RB", name="RB")
                nc.scalar.activation(RB[:], FB[:], AF.Copy)

                # Matmuls: stage-A degrees first (overlap with cast B).
                for stage, R, degs in (("A", RA, DEG_A), ("B", RB, DEG_B)):
                    for h in range(2):
                        ic = ic0 + h
                        for bs in range(N_BS):
                            for d, slot in degs.items():
                                nc.tensor.matmul(
                                    ps[bs][:],
                                    lhsT=R[:, slot * PW + h * B_BLK + bs * 128:
                                           slot * PW + h * B_BLK + (bs + 1) * 128],
                                    rhs=c_tiles[d - 1][:, ic, :],
                                    start=False,
                                    stop=(pair == 1 and stage == "B"
                                          and h == 1 and d == DEG),
                                )

            for bs in range(N_BS):
                o = opool.tile([128, O_DIM], dt.float32, tag="o")
                nc.scalar.activation(o[:], ps[bs][:], AF.Copy)
                nc.sync.dma_start(
                    out=y_out[b0 + bs * 128: b0 + (bs + 1) * 128, :], in_=o[:]
                )

    nc.compile()
    return nc


def _get_program():
    if "nc" not in _CACHE:
        _CACHE["nc"] = _build_program()
    return _CACHE["nc"]


def _prep_inputs(x, cheby_coeffs):
    x = np.ascontiguousarray(x, dtype=np.float32)
    c = np.ascontiguousarray(cheby_coeffs, dtype=np.float32)
    cd = np.ascontiguousarray(np.transpose(c, (2, 0, 1))[1:DEG + 1])  # [8, I, O]
    bias = c[:, :, 0].sum(axis=0, dtype=np.float64).astype(np.float32)[None, :]
    ones = np.ones((1, 128), dtype=np.float32)
    in_maps = []
    for core in range(N_CORES):
        xs = x[core * B_CORE:(core + 1) * B_CORE]          # [2048, I]
        xt = np.ascontiguousarray(xs.T)                     # [I, 2048]
        in_maps.append({"xt": xt, "cd": cd, "bias": bias, "ones": ones})
    return in_maps


def run(x, cheby_coeffs, trace=False, **trace_kwargs):
    nc = _get_program()
    in_maps = _prep_inputs(x, cheby_coeffs)
    res = run_bass_kernel_spmd(
        nc, in_maps, list(range(N_CORES)), trace=trace, **trace_kwargs
    )
    y = np.concatenate([res.results[i]["y"] for i in range(N_CORES)], axis=0)
    return y, res


def kernel(x, cheby_coeffs):
    y, _ = run(x, cheby_coeffs)
    return y


# revision 23
# speedup vs baseline: 1.0382x; 1.0020x over previous
"""ChebyKAN layer on 8 TRN2 NeuronCores (data-parallel over batch).

y[b,o] = sum_{i,d} T_d(tanh(x[b,i])) * C[i,o,d],  d = 0..8

Device algorithm (per core, batch shard of 2048 rows, blocks of 512):
  - T_0 = 1 is folded into a host-computed bias; a K=1 ones x bias matmul
    seeds each PSUM accumulation group with it.
  - t = tanh(x) on ACT (basis laid out transposed: [i_chunk=128 part, batch
    free], two i-chunks batched per op -> [128, 1024] tiles).
  - Chebyshev basis in fp32 via product identities split across engines:
    DVE:    T2 = 2t^2-1, T4 = 2T2^2-1, T6 = 2T2*T4-T2, T8 = 2T2*T6-T4
    GpSimd: T3 = t*(2T2-1), T5 = (2T2)*T3 - t, T7 = (2T2)*T5 - T3
  - Basis rounded fp32 -> fp16 in two wide ACT casts (stage A: t,T2,T3,T4;
    stage B: T5..T8); fp16 keeps 11 mantissa bits (like f32r) but the
    2-byte LDWEIGHTS hides under the matmuls, unlike 4-byte f32r.
  - PE: stationary = fp16 basis slice [128,128], moving = fp16 coefficient
    chunk [i=128, o=512], accumulated over (d, i_chunk) into PSUM
    [b=128, o=512]; coefficients are gpsimd cast-DMA'd fp32 -> fp16 once
    and stay resident (4.2 MB).
  - PSUM evacuated with an ACT copy, stored to DRAM over sync-engine DMA.

Measured on trn2 (8 cores, NTFF profile): ~202 us HW exec, relative error
~2.5e-4 vs the fp32 jax reference (fp16 rounding of basis + coefficients).

Inputs arrive FULL; sharding/transpose/reorder happen on the host here.
"""

import numpy as np

import concourse.bacc as bacc
import concourse.tile as tile
from concourse import mybir
from concourse.bass_utils import run_bass_kernel_spmd

dt = mybir.dt

BATCH = 16384
I_DIM = 512
O_DIM = 512
DEG = 8            # d = 1..8 on device; d=0 via bias
N_CORES = 8
B_CORE = BATCH // N_CORES      # 2048
B_BLK = 512                    # batch rows per block
N_BLK = B_CORE // B_BLK        # 4
N_IC = I_DIM // 128            # 4 input chunks
N_BS = B_BLK // 128            # 4 psum row-tiles per block

_CACHE = {}


def _build_program():
    from contextlib import ExitStack

    AF = mybir.ActivationFunctionType
    OP = mybir.AluOpType

    nc = bacc.Bacc(num_swdge_queues=4)
    xt_in = nc.declare_dram_parameter("xt", [I_DIM, B_CORE], dt.float32, isOutput=False)
    cd_in = nc.declare_dram_parameter("cd", [DEG, I_DIM, O_DIM], dt.float32, isOutput=False)
    bias_in = nc.declare_dram_parameter("bias", [1, O_DIM], dt.float32, isOutput=False)
    ones_in = nc.declare_dram_parameter("ones", [1, 128], dt.float32, isOutput=False)
    y_out = nc.declare_dram_parameter("y", [B_CORE, O_DIM], dt.float32, isOutput=True)

    # Two i-chunks are batched per elementwise op: every chain op works
    # on [128, 2*B_BLK] = [128, 1024].  Degrees live in slots:
    #   stage A slots: 0:t 1:T2 2:T3 3:T4   -> cast A
    #   stage B slots: 0:T5 1:T6 2:T7 3:T8  -> cast B
    PW = 2 * B_BLK            # 1024, pair width
    DEG_A = {1: 0, 2: 1, 3: 2, 4: 3}
    DEG_B = {5: 0, 6: 1, 7: 2, 8: 3}

    with tile.TileContext(nc) as tc, ExitStack() as ctx:
        cpool = ctx.enter_context(tc.tile_pool(name="cpool", bufs=1))
        xpool = ctx.enter_context(tc.tile_pool(name="xpool", bufs=2))
        fpool = ctx.enter_context(tc.tile_pool(name="fpool", bufs=2))
        rpool = ctx.enter_context(tc.tile_pool(name="rpool", bufs=2))
        mvpool = ctx.enter_context(tc.tile_pool(name="mvpool", bufs=2))
        s2pool = ctx.enter_context(tc.tile_pool(name="s2pool", bufs=1))
        mgpool = ctx.enter_context(tc.tile_pool(name="mgpool", bufs=2))
        opool = ctx.enter_context(tc.tile_pool(name="opool", bufs=2))
        pspool = ctx.enter_context(tc.tile_pool(name="pspool", bufs=8, space="PSUM"))

        # Bias (T_0 term) and a ones row: K=1 matmul seeds PSUM with the bias.
        bias_t = cpool.tile([1, O_DIM], dt.float16, tag="bias")
        nc.gpsimd.dma_start(out=bias_t[:], in_=bias_in[:])
        ones_t = cpool.tile([1, 128], dt.float16, tag="ones")
        nc.gpsimd.dma_start(out=ones_t[:], in_=ones_in[:])

        # Coefficients: one wide cast-DMA (fp32 -> f32r) per degree, resident.
        c_tiles = {}
        for d in range(DEG):
            c = cpool.tile([128, N_IC, O_DIM], dt.float16, tag=f"c{d}", name=f"c{d}")
            nc.gpsimd.dma_start(
                out=c[:],
                in_=cd_in[d].rearrange("(ic p) o -> p ic o", p=128),
            )
            c_tiles[d] = c

        for blk in range(N_BLK):
            b0 = blk * B_BLK
            ps = []
            for bs in range(N_BS):
                p = pspool.tile([128, O_DIM], dt.float32, tag="ps", name="ps")
                nc.tensor.matmul(
                    p[:], lhsT=ones_t[:], rhs=bias_t[:], start=True, stop=False
                )
                ps.append(p)
            for pair in range(N_IC // 2):
                ic0 = pair * 2
                xt = xpool.tile([128, PW], dt.float32, tag="xt")
                for h in range(2):
                    ic = ic0 + h
                    nc.sync.dma_start(
                        out=xt[:, h * B_BLK:(h + 1) * B_BLK],
                        in_=xt_in[ic * 128:(ic + 1) * 128, b0:b0 + B_BLK],
                    )
                FA = fpool.tile([128, 4 * PW], dt.float32, tag="FA", name="FA")
                FB = fpool.tile([128, 4 * PW], dt.float32, tag="FB", name="FB")

                def sa(i):
                    return FA[:, i * PW:(i + 1) * PW]

                def sb(i):
                    return FB[:, i * PW:(i + 1) * PW]

                t, s, T3, T4 = sa(0), sa(1), sa(2), sa(3)
                T5, T6, T7, T8 = sb(0), sb(1), sb(2), sb(3)

                nc.scalar.activation(t, xt[:], AF.Tanh)

                # DVE: T2, preps, even chain (tensor_scalar runs in 2x mode)
                m2 = mvpool.tile([128, PW], dt.float32, tag="mv", name="m2")
                nc.vector.scalar_tensor_tensor(m2[:], t, 2.0, t, OP.mult, OP.mult)
                nc.vector.tensor_scalar_sub(s, m2[:], 1.0)
                s2 = s2pool.tile([128, PW], dt.float32, tag="s2", name="s2")
                nc.vector.tensor_scalar_mul(s2[:], s, 2.0)
                w = s2pool.tile([128, PW], dt.float32, tag="w", name="w")
                nc.vector.tensor_scalar(w[:], s, 2.0, 1.0, OP.mult, OP.subtract)
                m4 = mvpool.tile([128, PW], dt.float32, tag="mv", name="m4")
                nc.vector.scalar_tensor_tensor(m4[:], s, 2.0, s, OP.mult, OP.mult)
                nc.vector.tensor_scalar_sub(T4, m4[:], 1.0)
                m6 = mvpool.tile([128, PW], dt.float32, tag="mv", name="m6")
                nc.vector.scalar_tensor_tensor(m6[:], T4, 2.0, s, OP.mult, OP.mult)
                nc.vector.tensor_sub(T6, m6[:], s)
                m8 = mvpool.tile([128, PW], dt.float32, tag="mv", name="m8")
                nc.vector.scalar_tensor_tensor(m8[:], T6, 2.0, s, OP.mult, OP.mult)
                nc.vector.tensor_sub(T8, m8[:], T4)

                # GpSimd: odd chain muls; final T7 subtract on DVE
                nc.gpsimd.tensor_mul(T3, t, w[:])
                m5 = mgpool.tile([128, PW], dt.float32, tag="mg", name="m5")
                nc.gpsimd.tensor_mul(m5[:], s2[:], T3)
                nc.gpsimd.tensor_sub(T5, m5[:], t)
                m7 = mgpool.tile([128, PW], dt.float32, tag="mg", name="m7")
                nc.gpsimd.tensor_mul(m7[:], s2[:], T5)
                nc.gpsimd.tensor_sub(T7, m7[:], T3)

                # Two-stage rounding casts fp32 -> fp16 on ACT.
                RA = rpool.tile([128, 4 * PW], dt.float16, tag="RA", name="RA")
                nc.scalar.activation(RA[:], FA[:], AF.Copy)
                RB = rpool.tile([128, 4 * PW], dt.float16, tag="RB", name="RB")
                nc.scalar.activation(RB[:], FB[:], AF.Copy)

                # Matmuls: stage-A degrees first (overlap with cast B).
                for stage, R, degs in (("A", RA, DEG_A), ("B", RB, DEG_B)):
                    for h in range(2):
                        ic = ic0 + h
                        for bs in range(N_BS):
                            for d, slot in degs.items():
                                nc.tensor.matmul(
                                    ps[bs][:],
                                    lhsT=R[:, slot * PW + h * B_BLK + bs * 128:
                                           slot * PW + h * B_BLK + (bs + 1) * 128],
                                    rhs=c_tiles[d - 1][:, ic, :],
                                    start=False,
                                    stop=(pair == 1 and stage == "B"
                                          and h == 1 and d == DEG),
                                )

            for bs in range(N_BS):
                o = opool.tile([128, O_DIM], dt.float32, tag="o")
                nc.scalar.activation(o[:], ps[bs][:], AF.Copy)
                nc.sync.dma_start(
                    out=y_out[b0 + bs * 128: b0 + (bs + 1) * 128, :], in_=o[:]
                )

    nc.compile()
    return nc


def _get_program():
    if "nc" not in _CACHE:
        _CACHE["nc"] = _build_program()
    return _CACHE["nc"]


def _prep_inputs(x, cheby_coeffs):
    x = np.ascontiguousarray(x, dtype=np.float32)
    c = np.ascontiguousarray(cheby_coeffs, dtype=np.float32)
    cd = np.ascontiguousarray(np.transpose(c, (2, 0, 1))[1:DEG + 1])  # [8, I, O]
    bias = c[:, :, 0].sum(axis=0, dtype=np.float64).astype(np.float32)[None, :]
    ones = np.ones((1, 128), dtype=np.float32)
    in_maps = []
    for core in range(N_CORES):
        xs = x[core * B_CORE:(core + 1) * B_CORE]          # [2048, I]
        xt = np.ascontiguousarray(xs.T)                     # [I, 2048]
        in_maps.append({"xt": xt, "cd": cd, "bias": bias, "ones": ones})
    return in_maps


def run(x, cheby_coeffs, trace=False, **trace_kwargs):
    nc = _get_program()
    in_maps = _prep_inputs(x, cheby_coeffs)
    res = run_bass_kernel_spmd(
        nc, in_maps, list(range(N_CORES)), trace=trace, **trace_kwargs
    )
    y = np.concatenate([res.results[i]["y"] for i in range(N_CORES)], axis=0)
    return y, res


def kernel(x, cheby_coeffs):
    y, _ = run(x, cheby_coeffs)
    return y
